# revision 1
# baseline (speedup 1.0000x reference)
"""Distributed kNN retrieval kernel for trn2 (8 NeuronCores).

Math: reference ranks candidates per query by cosine distance
1 - dot/(|q||m|); query norm is constant per row, so ranking is by
dot(q, m)/|m|.  Host pre-normalizes matching rows (fp64 norms), so the
device only computes S = Q @ Mn^T, takes per-query top-4, and averages
synth rows.

Distribution: candidates (100000) row-sharded 12500/core (padded 12800);
synth column-sharded 128 features/core.  Per core:
  bf16x3 matmul (Qh.Mh + Qh.Ml + Ql.Mh accumulated in fp32 PSUM)
  -> per-block top-8 (nc.vector.max/max_index) -> local top-4
  -> AllGather (2048 x 8 fp32) -> replicated global top-4 merge
  -> indirect-DMA gather of this core's 128 synth columns -> mean
  -> output [2048, 128] slice; host concatenates along features.
"""
import sys

import numpy as np

sys.path.insert(0, "/opt/trn_rl_repo")
import ml_dtypes  # noqa: E402
import concourse.bacc as bacc  # noqa: E402
import concourse.bass as bass  # noqa: E402
import concourse.mybir as mybir  # noqa: E402
import concourse.tile as tile  # noqa: E402
from concourse.bass import IndirectOffsetOnAxis  # noqa: E402
from concourse.bass_utils import run_bass_kernel_spmd  # noqa: E402

NCORES = 8
FRM = 2048          # queries
F = 1024            # features
C = 100000          # candidates
SHARD = C // NCORES         # 12500
CW = 500                    # candidate-chunk width (25*500 = 12500, no pad)
KCH = F // 128              # 8 contraction chunks
NQT = FRM // 128            # 16 query tiles
NCCH = SHARD // CW          # 25 candidate chunks
BLOCKS = [(b * 3, 3) for b in range(8)] + [(24, 1)]  # (cchunk0, n) -> 9 blocks
NB = len(BLOCKS)
NCAND = NB * 8              # 72 local candidates per query
FSL = F // NCORES           # 128 synth feature columns per core
SPLITS = [(0, 4), (4, 4), (8, 4), (12, 4)]  # (start, n) qtile groups

BF16 = mybir.dt.bfloat16
F32 = mybir.dt.float32
U32 = mybir.dt.uint32
I32 = mybir.dt.int32


def build():
    nc = bacc.Bacc(num_devices=NCORES)
    QHL = nc.declare_dram_parameter("qhl", [128, 2 * KCH * FRM], BF16, isOutput=False)
    MHL = nc.declare_dram_parameter("mhl", [NCCH, 128, KCH * 2 * CW], BF16, isOutput=False)
    SYN = nc.declare_dram_parameter("syn", [C, FSL], F32, isOutput=False)
    COFF = nc.declare_dram_parameter("coff", [128, 1], F32, isOutput=False)
    OUT = nc.declare_dram_parameter("out", [FRM, FSL], F32, isOutput=True)

    HI_OFF = KCH * FRM  # bf16 column offset of the lo half in QHL

    with tile.TileContext(nc) as tc:
        with tc.tile_pool(name="cst", bufs=1) as cst, \
             tc.tile_pool(name="mpool", bufs=5) as mpool, \
             tc.tile_pool(name="blk", bufs=3) as blk, \
             tc.tile_pool(name="cand", bufs=2) as cand, \
             tc.tile_pool(name="sm", bufs=4) as sm, \
             tc.tile_pool(name="gat", bufs=3) as gat, \
             tc.tile_pool(name="psw", bufs=1, space="PSUM") as psw, \
             tc.tile_pool(name="ps", bufs=6, space="PSUM") as ps, \
             tc.tile_pool(name="dram", bufs=4, space="DRAM") as dram:

            # tiny warmup weights + first-qtile weights first, so PE and the
            # first matmul group start as soon as possible
            QB = KCH * 128  # 1024 cols per qtile block
            wt = cst.tile([128, 128], BF16)
            nc.sync.dma_start(out=wt[:], in_=QHL[:, :128])
            qhl = cst.tile([128, 2 * KCH * FRM], BF16)
            nc.sync.dma_start(out=qhl[:, :QB], in_=QHL[:, :QB])
            nc.sync.dma_start(out=qhl[:, HI_OFF:HI_OFF + QB],
                              in_=QHL[:, HI_OFF:HI_OFF + QB])
            coff = cst.tile([128, 1], F32)
            nc.sync.dma_start(out=coff[:], in_=COFF[:])

            pw = psw.tile([128, 128], F32)
            nc.tensor.matmul(out=pw[:], lhsT=wt[:], rhs=wt[:],
                             start=True, stop=True)

            def qs(hl, k, t):
                base = hl * HI_OFF + t * KCH * 128 + k * 128
                return qhl[:, base:base + 128]

            for s, (q0, QTPS) in enumerate(SPLITS):
                qtiles = range(q0, q0 + QTPS)
                valsall = cand.tile([128, 4 * NCAND], F32, tag="valsall")
                idxall = cand.tile([128, 4 * NCAND], F32, tag="idxall")

                # ---- phase 1+2: scores + per-block top-8 ----
                for b, (c0, nch) in enumerate(BLOCKS):
                    mts = []
                    for ci in range(nch):
                        mt = mpool.tile([128, KCH * 2 * CW], BF16, tag="mt")
                        nc.sync.dma_start(out=mt[:], in_=MHL[c0 + ci])
                        mts.append(mt)
                    if s == 0 and b == 0:
                        # rest of the Q weights, after the urgent first tiles
                        nc.sync.dma_start(out=qhl[:, QB:HI_OFF],
                                          in_=QHL[:, QB:HI_OFF])
                        nc.sync.dma_start(out=qhl[:, HI_OFF + QB:],
                                          in_=QHL[:, HI_OFF + QB:])
                    for tl, t in enumerate(qtiles):
                        sblk = blk.tile([128, 3 * CW], F32, tag="sblk")
                        for ci in range(nch):
                            p = ps.tile([128, CW], F32, tag="p")
                            i = 0
                            for hq, hm in ((0, 0), (0, 1), (1, 0)):
                                for k in range(KCH):
                                    ms = mts[ci][:, (k * 2 + hm) * CW:
                                                 (k * 2 + hm + 1) * CW]
                                    nc.tensor.matmul(out=p[:], lhsT=qs(hq, k, t),
                                                     rhs=ms,
                                                     start=(i == 0), stop=(i == 23))
                                    i += 1
                            nc.scalar.copy(out=sblk[:, ci * CW:(ci + 1) * CW], in_=p[:])
                        sb_v = sblk[:, :nch * CW]
                        vsl = valsall[:, tl * NCAND + b * 8: tl * NCAND + b * 8 + 8]
                        isl = idxall[:, tl * NCAND + b * 8: tl * NCAND + b * 8 + 8]
                        bi = sm.tile([128, 8], U32, tag="bi")
                        bif = sm.tile([128, 8], F32, tag="bif")
                        nc.vector.max(out=vsl, in_=sb_v)
                        nc.vector.max_index(out=bi[:], in_max=vsl, in_values=sb_v)
                        nc.vector.tensor_copy(out=bif[:], in_=bi[:])  # u32 -> f32
                        # global candidate id = pos + CW*c0 + core_offset
                        nc.vector.tensor_scalar(
                            out=isl, in0=bif[:], scalar1=coff[:, 0:1],
                            scalar2=float(CW * c0),
                            op0=mybir.AluOpType.add, op1=mybir.AluOpType.add)

                # ---- phase 2b: local top-4 -> cc_in ----
                cc_in = dram.tile([QTPS * 128, 8], F32, tag="ccin")
                cc_out = dram.tile([NCORES * QTPS * 128, 8], F32, tag="ccout")
                for tl, t in enumerate(qtiles):
                    va = valsall[:, tl * NCAND:(tl + 1) * NCAND]
                    ia = idxall[:, tl * NCAND:(tl + 1) * NCAND]
                    lv = sm.tile([128, 8], F32, tag="lv")
                    loc = sm.tile([128, 8], F32, tag="loc")
                    nc.vector.max(out=lv[:], in_=va)
                    nc.vector.tensor_copy(out=loc[:, 0:4], in_=lv[:, 0:4])
                    eq = sm.tile([128, 4 * NCAND], F32, tag="eq")
                    eq3 = eq[:].rearrange("p (j n) -> p j n", j=4)
                    nc.vector.tensor_tensor(
                        out=eq3, in0=va.unsqueeze(1).to_broadcast([128, 4, NCAND]),
                        in1=lv[:, 0:4].unsqueeze(2).to_broadcast([128, 4, NCAND]),
                        op=mybir.AluOpType.is_equal)
                    nc.vector.tensor_tensor(
                        out=eq3, in0=eq3,
                        in1=ia.unsqueeze(1).to_broadcast([128, 4, NCAND]),
                        op=mybir.AluOpType.mult)
                    nc.vector.tensor_reduce(
                        out=loc[:, 4:8], in_=eq3,
                        axis=mybir.AxisListType.X, op=mybir.AluOpType.max)
                    nc.sync.dma_start(out=cc_in[tl * 128:(tl + 1) * 128, :], in_=loc[:])

                # ---- phase 3: AllGather candidates for this split ----
                nc.gpsimd.collective_compute(
                    "AllGather", mybir.AluOpType.bypass,
                    replica_groups=[list(range(NCORES))],
                    ins=[cc_in.opt()], outs=[cc_out.opt()])

                # ---- phase 4+5: global merge, gather-accumulate, mean ----
                cc_view = cc_out[:].rearrange("(r q) e -> q r e", r=NCORES)
                for tl, t in enumerate(qtiles):
                    cands = sm.tile([128, NCORES * 8], F32, tag="cands")
                    nc.sync.dma_start(
                        out=cands[:].rearrange("p (r e) -> p r e", r=NCORES),
                        in_=cc_view[tl * 128:(tl + 1) * 128])
                    cv = sm.tile([128, 32], F32, tag="cv")
                    cvi = sm.tile([128, 32], F32, tag="cvi")
                    c3 = cands[:].rearrange("p (r e) -> p r e", r=NCORES)
                    nc.vector.tensor_copy(out=cv[:].rearrange("p (r e) -> p r e", r=8),
                                          in_=c3[:, :, 0:4])
                    nc.vector.tensor_copy(out=cvi[:].rearrange("p (r e) -> p r e", r=8),
                                          in_=c3[:, :, 4:8])
                    gv = sm.tile([128, 8], F32, tag="gv")
                    gif = sm.tile([128, 4], F32, tag="gif")
                    nc.vector.max(out=gv[:], in_=cv[:])
                    eq2 = sm.tile([128, 4 * 32], F32, tag="eq2")
                    e3 = eq2[:].rearrange("p (j n) -> p j n", j=4)
                    nc.vector.tensor_tensor(
                        out=e3, in0=cv[:].unsqueeze(1).to_broadcast([128, 4, 32]),
                        in1=gv[:, 0:4].unsqueeze(2).to_broadcast([128, 4, 32]),
                        op=mybir.AluOpType.is_equal)
                    nc.vector.tensor_tensor(
                        out=e3, in0=e3,
                        in1=cvi[:].unsqueeze(1).to_broadcast([128, 4, 32]),
                        op=mybir.AluOpType.mult)
                    nc.vector.tensor_reduce(
                        out=gif[:], in_=e3,
                        axis=mybir.AxisListType.X, op=mybir.AluOpType.max)
                    gii = sm.tile([128, 4], I32, tag="gii")
                    nc.vector.tensor_copy(out=gii[:], in_=gif[:])  # f32 -> i32
                    gbuf = gat.tile([128, FSL], F32, tag="gbuf")
                    nc.vector.memset(gbuf[:], 0.0)
                    for j in range(4):
                        nc.gpsimd.indirect_dma_start(
                            out=gbuf[:], out_offset=None,
                            in_=SYN[:],
                            in_offset=IndirectOffsetOnAxis(ap=gii[:, j:j + 1], axis=0),
                            compute_op=mybir.AluOpType.add)
                    nc.vector.tensor_scalar_mul(gbuf[:], gbuf[:], 0.25)
                    nc.sync.dma_start(out=OUT[t * 128:(t + 1) * 128, :], in_=gbuf[:])

    nc.compile()
    return nc


# ---------------- host side ----------------

def _split_bf16(x):
    hi = x.astype(ml_dtypes.bfloat16)
    lo = (x - hi.astype(np.float32)).astype(ml_dtypes.bfloat16)
    return hi, lo


def prepare_inputs(query_seq, matching_set, synth_set):
    """Returns per-core in_maps."""
    q = np.asarray(query_seq, dtype=np.float32)
    m = np.asarray(matching_set, dtype=np.float32)
    syn = np.asarray(synth_set, dtype=np.float32)

    # normalize matching rows with fp64 norms
    norms = np.linalg.norm(m.astype(np.float64), axis=1, keepdims=True)
    mn = (m / norms).astype(np.float32)

    # Q^T packed [128, 2*KCH*FRM]
    qt = np.ascontiguousarray(q.T)                       # [1024, 2048]
    qh, ql = _split_bf16(qt)
    def pack_q(a):
        return a.reshape(KCH, 128, NQT, 128).transpose(1, 2, 0, 3).reshape(128, KCH * FRM)
    qhl = np.concatenate([pack_q(qh), pack_q(ql)], axis=1).copy()

    in_maps = []
    for core in range(NCORES):
        shard = mn[core * SHARD:(core + 1) * SHARD]      # [12500, 1024]
        mt = np.ascontiguousarray(shard.T)               # [1024, 12500]
        mh, ml = _split_bf16(mt)
        # [k,p,c,n] -> [c,p,k,hl,n] -> [25, 128, 8000]
        mh4 = mh.reshape(KCH, 128, NCCH, CW).transpose(2, 1, 0, 3)
        ml4 = ml.reshape(KCH, 128, NCCH, CW).transpose(2, 1, 0, 3)
        mhl = np.stack([mh4, ml4], axis=3).reshape(NCCH, 128, KCH * 2 * CW).copy()

        in_maps.append({
            "qhl": qhl,
            "mhl": mhl,
            "syn": np.ascontiguousarray(syn[:, core * FSL:(core + 1) * FSL]),
            "coff": np.full((128, 1), float(core * SHARD), dtype=np.float32),
        })
    return in_maps


_NC_CACHE = {}


def run(query_seq, matching_set, synth_set, topk=4, trace=False):
    assert int(topk) == 4, f"kernel is specialized for topk=4, got {topk}"
    in_maps = prepare_inputs(query_seq, matching_set, synth_set)
    if "nc" not in _NC_CACHE:
        _NC_CACHE["nc"] = build()
    nc = _NC_CACHE["nc"]
    res = run_bass_kernel_spmd(nc, in_maps, core_ids=list(range(NCORES)),
                               trace=trace)
    out = np.concatenate([res.results[i]["out"] for i in range(NCORES)], axis=1)
    return out.astype(np.float32), res


def kernel(**inputs):
    topk = inputs.get("topk", 4)
    try:
        topk = int(np.asarray(topk))
    except Exception:
        topk = int(topk)
    out, _ = run(inputs["query_seq"], inputs["matching_set"],
                 inputs["synth_set"], topk)
    return out
